# revision 10
# baseline (speedup 1.0000x reference)
"""Guided filter (r=40, eps=1e-3) on 8 Trainium2 NeuronCores.

Sharding: pure data-parallel over the batch dim (8 batches -> 8 cores).
Each core processes 3 channel-images of 512x512.

All-matmul box2d: each direction is a banded matmul on TensorE (data
stationary, 0/1*(1/n) band moving) that box-filters one axis and
transposes the layout. Both bands carry bf16(1/n(out)) so V+W yields the
fully normalized mean directly - the elementwise stage then has no
per-partition scalars and runs as full-width [128, 2048] bf16 ops in
DVE high-perf mode. eps folds into mean_II via rank-1 (K=1) matmuls.
Each pass accumulates into one 4-bank PSUM tile, drained by a single
batched copy/cast, split across ACT/DVE/Pool to balance engine load.
"""

import os
import sys
import numpy as np
import ml_dtypes
from contextlib import ExitStack

sys.path.insert(0, "/opt/trn_rl_repo")

import concourse.bass as bass
import concourse.tile as tile
from concourse import bacc, mybir
from concourse.bass_utils import run_bass_kernel_spmd

F32 = mybir.dt.float32
BF16 = mybir.dt.bfloat16
ALU = mybir.AluOpType

R = 40
EPS = 1e-3
HW_ = 512
NB = 4
CH = 3
P = 128
NCORES = 8
FW = NB * HW_  # 2048


def _maybe_patch_ldw_opt():
    """Optionally flip walrus --enable-ldw-opt (pipelines weight loads)."""
    if os.environ.get("BASS_LDW_OPT") != "1":
        return
    import concourse.bass_utils as bu
    if getattr(bu, "_ldw_patched", False):
        return
    orig = bu.run_command

    def patched(argv, **kwargs):
        argv = ["--enable-ldw-opt=true" if a == "--enable-ldw-opt=false" else a
                for a in argv]
        return orig(argv, **kwargs)

    bu.run_command = patched
    bu._ldw_patched = True


def _band_range(c):
    n0 = max(0, P * c - R)
    n1 = min(HW_, P * c + P + R)
    return n0, n1


def make_consts():
    idx = np.arange(HW_)
    n1d = (np.minimum(idx + R, HW_ - 1) - np.maximum(idx - R, 0) + 1).astype(np.float64)
    inv_n = 1.0 / n1d
    mask = (np.abs(idx[:, None] - idx[None, :]) <= R)
    bandH = (mask * inv_n[None, :]).astype(ml_dtypes.bfloat16)
    bandH = np.ascontiguousarray(
        bandH.reshape(NB, P, HW_).transpose(1, 0, 2).reshape(P, NB * HW_))
    eps1 = np.full((1, HW_), EPS, dtype=ml_dtypes.bfloat16)
    ones = np.ones((1, HW_), dtype=ml_dtypes.bfloat16)
    return {"bandH": bandH, "eps1": eps1, "ones": ones}


def _img_view(dram_ap, c):
    return dram_ap[c].rearrange("(hb hp) w -> hp hb w", hp=P)


def _sb3(t):
    return t[:].rearrange("p (hb w) -> p hb w", w=HW_)


def build_model():
    nc = bacc.Bacc("TRN2", target_bir_lowering=False, debug=False,
                   num_devices=NCORES)
    I_d = nc.dram_tensor("I", [CH, HW_, HW_], F32, kind="ExternalInput").ap()
    p_d = nc.dram_tensor("p", [CH, HW_, HW_], F32, kind="ExternalInput").ap()
    bandH_d = nc.dram_tensor("bandH", [P, FW], BF16, kind="ExternalInput").ap()
    eps1_d = nc.dram_tensor("eps1", [1, HW_], BF16, kind="ExternalInput").ap()
    ones_d = nc.dram_tensor("ones", [1, HW_], BF16, kind="ExternalInput").ap()
    out_d = nc.dram_tensor("out", [CH, HW_, HW_], F32, kind="ExternalOutput").ap()

    with tile.TileContext(nc) as tc:
        with ExitStack() as ctx:
            build_kernel(ctx, tc, I_d, p_d, out_d, bandH_d, eps1_d, ones_d)
    nc.compile()
    return nc


def build_kernel(ctx, tc, I_d, p_d, out_d, bandH_d, eps1_d, ones_d):
    nc = tc.nc

    consts = ctx.enter_context(tc.tile_pool(name="consts", bufs=1))
    bandH = consts.tile_from(bandH_d)
    eps1 = consts.tile_from(eps1_d)
    ones = consts.tile_from(ones_d)

    pIn = ctx.enter_context(tc.tile_pool(name="inp", bufs=2))
    pBf = ctx.enter_context(tc.tile_pool(name="ibf", bufs=2))
    pY = ctx.enter_context(tc.tile_pool(name="ymid", bufs=1))
    pM = ctx.enter_context(tc.tile_pool(name="means", bufs=1))
    pS2 = ctx.enter_context(tc.tile_pool(name="st2", bufs=1))
    pOut = ctx.enter_context(tc.tile_pool(name="outp", bufs=2))
    pV = ctx.enter_context(tc.tile_pool(name="psv", bufs=1, space="PSUM"))
    pW = ctx.enter_context(tc.tile_pool(name="psw", bufs=1, space="PSUM"))

    def vpass(src_bf, Vb):
        """Box over h (partition) + transpose: src [hp,(hb,w)] -> Vb
        [wp,(wb,h)] in a single 4-bank psum tile."""
        for i in range(NB):
            for j in range(NB):
                n0, n1 = _band_range(j)
                nc.tensor.matmul(
                    Vb[:, HW_ * i + n0: HW_ * i + n1],
                    lhsT=src_bf[:, j * HW_ + i * P: j * HW_ + i * P + P],
                    rhs=bandH[:, j * HW_ + n0: j * HW_ + n1],
                    start=(j == 0), stop=(j == NB - 1))

    def wpass(y_bf, Wb, add_eps=False):
        """Box over w (partition) + transpose back: y [wp,(wb,h)] -> Wb
        [hp,(hb,w)]."""
        for j in range(NB):
            for i in range(NB):
                m0, m1 = _band_range(i)
                last = (i == NB - 1) and not add_eps
                nc.tensor.matmul(
                    Wb[:, HW_ * j + m0: HW_ * j + m1],
                    lhsT=y_bf[:, i * HW_ + j * P: i * HW_ + j * P + P],
                    rhs=bandH[:, i * HW_ + m0: i * HW_ + m1],
                    start=(i == 0), stop=last)
            if add_eps:
                nc.tensor.matmul(
                    Wb[:, HW_ * j: HW_ * (j + 1)],
                    lhsT=eps1[:1, 0:P], rhs=ones[:1, :],
                    start=False, stop=True)

    def stt(eng, out, in0, in1, op):
        eng.scalar_tensor_tensor(out, in0, 0.0, in1, op0=ALU.bypass, op1=op)

    def copy_eng(which, dst, src):
        if which == "dve":
            nc.vector.tensor_copy(dst, src)
        else:
            nc.scalar.copy(dst, src)

    for c in range(CH):
        I_f = pIn.tile([P, FW], F32, tag="If")
        p_f = pIn.tile([P, FW], F32, tag="pf")
        nc.sync.dma_start(_sb3(I_f), _img_view(I_d, c))
        nc.sync.dma_start(_sb3(p_f), _img_view(p_d, c))

        I_bf = pBf.tile([P, FW], BF16, tag="Ibf")
        p_bf = pBf.tile([P, FW], BF16, tag="pbf")
        Ip_bf = pBf.tile([P, FW], BF16, tag="Ipbf")
        II_bf = pBf.tile([P, FW], BF16, tag="IIbf")
        nc.gpsimd.tensor_copy(I_bf[:], I_f[:])
        nc.gpsimd.tensor_copy(p_bf[:], p_f[:])
        nc.vector.tensor_mul(Ip_bf[:], I_bf[:], p_bf[:])
        nc.scalar.square(II_bf[:], I_f[:])

        # V+W passes per tensor; drain each 4-bank psum with one batched op
        yI = pY.tile([P, FW], BF16, tag="yI")
        yp = pY.tile([P, FW], BF16, tag="yp")
        yIp = pY.tile([P, FW], BF16, tag="yIp")
        yII = pY.tile([P, FW], BF16, tag="yII")
        PIb = pM.tile([P, FW], BF16, tag="PIb")
        PPb = pM.tile([P, FW], BF16, tag="PPb")
        PIpb = pM.tile([P, FW], BF16, tag="PIpb")
        PIIb = pM.tile([P, FW], BF16, tag="PIIb")
        for src, y, Pb, ycp, pcp, epsf in (
                (I_bf, yI, PIb, "dve", "act", False),
                (p_bf, yp, PPb, "act", "act", False),
                (Ip_bf, yIp, PIpb, "dve", "dve", False),
                (II_bf, yII, PIIb, "act", "dve", True)):
            Vb = pV.tile([P, FW], F32, tag="V")
            vpass(src, Vb)
            copy_eng(ycp, y[:], Vb[:])
            Wb = pW.tile([P, FW], F32, tag="W")
            wpass(y, Wb, add_eps=epsf)
            copy_eng(pcp, Pb[:], Wb[:])

        # stage2: plain bf16 elementwise, no normalization scalars
        u = pS2.tile([P, FW], BF16, tag="u")
        cov = pS2.tile([P, FW], BF16, tag="cov")
        sq = pS2.tile([P, FW], BF16, tag="sq")
        den = pS2.tile([P, FW], F32, tag="den")
        rcp = pS2.tile([P, FW], F32, tag="rcp")
        a_bf = pS2.tile([P, FW], BF16, tag="abf")
        t1 = pS2.tile([P, FW], BF16, tag="t1")
        b_bf = pS2.tile([P, FW], BF16, tag="bbf")
        stt(nc.vector, u[:], PIb[:], PPb[:], ALU.mult)
        stt(nc.vector, cov[:], PIpb[:], u[:], ALU.subtract)
        stt(nc.vector, sq[:], PIb[:], PIb[:], ALU.mult)
        stt(nc.vector, den[:], PIIb[:], sq[:], ALU.subtract)
        nc.vector.reciprocal_approx_fast(rcp[:], den[:])
        stt(nc.vector, a_bf[:], cov[:], rcp[:], ALU.mult)
        stt(nc.vector, t1[:], a_bf[:], PIb[:], ALU.mult)
        stt(nc.vector, b_bf[:], PPb[:], t1[:], ALU.subtract)

        # stage3: box2d(a), box2d(b), combine
        ya = pY.tile([P, FW], BF16, tag="ya")
        yb = pY.tile([P, FW], BF16, tag="yb")
        Rab = pM.tile([P, FW], BF16, tag="Rab")
        Rbb = pM.tile([P, FW], BF16, tag="Rbb")
        for src, y, Rb_t, ycp, pcp in ((a_bf, ya, Rab, "dve", "act"),
                                       (b_bf, yb, Rbb, "act", "dve")):
            Vb = pV.tile([P, FW], F32, tag="V")
            vpass(src, Vb)
            copy_eng(ycp, y[:], Vb[:])
            Wb = pW.tile([P, FW], F32, tag="W")
            wpass(y, Wb)
            copy_eng(pcp, Rb_t[:], Wb[:])

        tt = pS2.tile([P, FW], BF16, tag="tt")
        out_t = pOut.tile([P, FW], F32, tag="out")
        stt(nc.vector, tt[:], Rab[:], I_bf[:], ALU.mult)
        stt(nc.vector, out_t[:], tt[:], Rbb[:], ALU.add)

        nc.sync.dma_start(_img_view(out_d, c), _sb3(out_t))


_NC_CACHE = None
LAST_RESULT = None


def _get_model():
    global _NC_CACHE
    if _NC_CACHE is None:
        _maybe_patch_ldw_opt()
        _NC_CACHE = build_model()
    return _NC_CACHE


def kernel(I, p):
    global LAST_RESULT
    I = np.asarray(I, dtype=np.float32)
    p = np.asarray(p, dtype=np.float32)
    B = I.shape[0]
    assert I.shape == (B, CH, HW_, HW_), I.shape
    nc = _get_model()
    consts = make_consts()
    in_maps = []
    for k in range(NCORES):
        m = {"I": np.ascontiguousarray(I[k]), "p": np.ascontiguousarray(p[k])}
        m.update(consts)
        in_maps.append(m)
    kwargs = {}
    if os.environ.get("BASS_TRACE_DIR"):
        kwargs["tmpdir"] = os.environ["BASS_TRACE_DIR"]
    res = run_bass_kernel_spmd(nc, in_maps, core_ids=list(range(NCORES)), **kwargs)
    LAST_RESULT = res
    out = np.stack([res.results[k]["out"] for k in range(NCORES)], axis=0)
    return out.astype(np.float32)


if __name__ == "__main__":
    rng = np.random.default_rng(0)
    I = rng.random((8, CH, HW_, HW_), dtype=np.float32)
    p = rng.random((8, CH, HW_, HW_), dtype=np.float32)
    out = kernel(I, p)
    print("out", out.shape, out.dtype, float(out.mean()))


# revision 11
# speedup vs baseline: 1.3669x; 1.3669x over previous
"""Guided filter (r=40, eps=1e-3) on 8 Trainium2 NeuronCores.

Sharding: pure data-parallel over the batch dim (8 batches -> 8 cores).
Each core processes 3 channel-images of 512x512.

box2d(x) = two banded matmuls on TensorE (data stationary, band moving);
each pass box-filters one axis and transposes the layout. Both bands
carry bf16(1/n(out)), so V+W produces fully normalized means directly
and the elementwise stage needs no normalization scalars. eps is added
to mean_II via a rank-1 (K=1) matmul per j-chunk. Elementwise stage
runs per j-chunk [128,512] reading PSUM directly where possible, with
the copy/square ops on ACT and tensor-tensor ops on DVE.
"""

import os
import sys
import numpy as np
import ml_dtypes
from contextlib import ExitStack

sys.path.insert(0, "/opt/trn_rl_repo")

import concourse.bass as bass
import concourse.tile as tile
from concourse import bacc, mybir
from concourse.bass_utils import run_bass_kernel_spmd

F32 = mybir.dt.float32
BF16 = mybir.dt.bfloat16
ALU = mybir.AluOpType

R = 40
EPS = 1e-3
HW_ = 512
NB = 4
CH = 3
P = 128
NCORES = 8
FW = NB * HW_


def _maybe_patch_ldw_opt():
    if os.environ.get("BASS_LDW_OPT") != "1":
        return
    import concourse.bass_utils as bu
    if getattr(bu, "_ldw_patched", False):
        return
    orig = bu.run_command

    def patched(argv, **kwargs):
        argv = ["--enable-ldw-opt=true" if a == "--enable-ldw-opt=false" else a
                for a in argv]
        return orig(argv, **kwargs)

    bu.run_command = patched
    bu._ldw_patched = True


def _band_range(c):
    n0 = max(0, P * c - R)
    n1 = min(HW_, P * c + P + R)
    return n0, n1


def make_consts():
    idx = np.arange(HW_)
    n1d = (np.minimum(idx + R, HW_ - 1) - np.maximum(idx - R, 0) + 1).astype(np.float64)
    inv_n = 1.0 / n1d
    mask = (np.abs(idx[:, None] - idx[None, :]) <= R)
    bandB = (mask * inv_n[None, :]).astype(ml_dtypes.bfloat16)
    bandB = np.ascontiguousarray(
        bandB.reshape(NB, P, HW_).transpose(1, 0, 2).reshape(P, NB * HW_))
    eps1 = np.full((1, HW_), EPS, dtype=ml_dtypes.bfloat16)
    ones = np.ones((1, HW_), dtype=ml_dtypes.bfloat16)
    return {"bandB": bandB, "eps1": eps1, "ones": ones}


def _img_view(dram_ap, c):
    return dram_ap[c].rearrange("(hb hp) w -> hp hb w", hp=P)


def _sb3(t):
    return t[:].rearrange("p (hb w) -> p hb w", w=HW_)


def build_model():
    nc = bacc.Bacc("TRN2", target_bir_lowering=False, debug=False,
                   num_devices=NCORES)
    I_d = nc.dram_tensor("I", [CH, HW_, HW_], F32, kind="ExternalInput").ap()
    p_d = nc.dram_tensor("p", [CH, HW_, HW_], F32, kind="ExternalInput").ap()
    bandB_d = nc.dram_tensor("bandB", [P, FW], BF16, kind="ExternalInput").ap()
    eps1_d = nc.dram_tensor("eps1", [1, HW_], BF16, kind="ExternalInput").ap()
    ones_d = nc.dram_tensor("ones", [1, HW_], BF16, kind="ExternalInput").ap()
    out_d = nc.dram_tensor("out", [CH, HW_, HW_], F32, kind="ExternalOutput").ap()

    with tile.TileContext(nc) as tc:
        with ExitStack() as ctx:
            build_kernel(ctx, tc, I_d, p_d, out_d, bandB_d, eps1_d, ones_d)
    nc.compile()
    return nc


def build_kernel(ctx, tc, I_d, p_d, out_d, bandB_d, eps1_d, ones_d):
    nc = tc.nc

    consts = ctx.enter_context(tc.tile_pool(name="consts", bufs=1))
    bandB = consts.tile_from(bandB_d)
    eps1 = consts.tile_from(eps1_d)
    ones = consts.tile_from(ones_d)

    pIf = ctx.enter_context(tc.tile_pool(name="If", bufs=2))
    pPf = ctx.enter_context(tc.tile_pool(name="Pf", bufs=2))
    pBf = ctx.enter_context(tc.tile_pool(name="ibf", bufs=1))
    pY = ctx.enter_context(tc.tile_pool(name="ymid", bufs=2))
    pAB = ctx.enter_context(tc.tile_pool(name="ab", bufs=2))
    pOut = ctx.enter_context(tc.tile_pool(name="outp", bufs=2))
    pT = ctx.enter_context(tc.tile_pool(name="tmps", bufs=2))
    pV = ctx.enter_context(tc.tile_pool(name="psv", bufs=2, space="PSUM"))
    pQ = ctx.enter_context(tc.tile_pool(name="psq", bufs=1, space="PSUM"))
    pRR = ctx.enter_context(tc.tile_pool(name="psr", bufs=1, space="PSUM"))

    def vpass(src_bf, dst_bf, eng="dve"):
        """Banded pass: box over partition axis + transpose."""
        for i in range(NB):
            ps = pV.tile([P, HW_], F32, tag="ps", name="ps")
            for j in range(NB):
                n0, n1 = _band_range(j)
                nc.tensor.matmul(
                    ps[:, n0:n1],
                    lhsT=src_bf[:, j * HW_ + i * P: j * HW_ + i * P + P],
                    rhs=bandB[:, j * HW_ + n0: j * HW_ + n1],
                    start=(j == 0), stop=(j == NB - 1))
            if eng == "dve":
                nc.vector.tensor_copy(dst_bf[:, i * HW_:(i + 1) * HW_], ps[:])
            else:
                nc.scalar.copy(dst_bf[:, i * HW_:(i + 1) * HW_], ps[:])

    def wpass_mm(src_bf, q_tile, j, add_eps=False):
        """W-direction banded MMs for output h-chunk j into q_tile."""
        for i in range(NB):
            m0, m1 = _band_range(i)
            last = (i == NB - 1) and not add_eps
            nc.tensor.matmul(
                q_tile[:, m0:m1],
                lhsT=src_bf[:, i * HW_ + j * P: i * HW_ + j * P + P],
                rhs=bandB[:, i * HW_ + m0: i * HW_ + m1],
                start=(i == 0), stop=last)
        if add_eps:
            nc.tensor.matmul(
                q_tile[:, :], lhsT=eps1[:1, 0:P], rhs=ones[:1, :],
                start=False, stop=True)

    for c in range(CH):
        I_f = pIf.tile([P, FW], F32, tag="If")
        p_f = pPf.tile([P, FW], F32, tag="pf")
        nc.sync.dma_start(_sb3(I_f), _img_view(I_d, c))
        nc.sync.dma_start(_sb3(p_f), _img_view(p_d, c))

        I_bf = pBf.tile([P, FW], BF16, tag="Ibf")
        p_bf = pBf.tile([P, FW], BF16, tag="pbf")
        Ip_bf = pBf.tile([P, FW], BF16, tag="Ipbf")
        II_bf = pBf.tile([P, FW], BF16, tag="IIbf")
        nc.scalar.copy(I_bf[:], I_f[:])
        nc.scalar.copy(p_bf[:], p_f[:])
        nc.vector.tensor_mul(Ip_bf[:], I_bf[:], p_bf[:])
        nc.scalar.square(II_bf[:], I_f[:])

        # stage 1: V-pass for the four tensors -> [w|h] bf16 mids
        yI = pY.tile([P, FW], BF16, tag="yI")
        yp = pY.tile([P, FW], BF16, tag="yp")
        yIp = pY.tile([P, FW], BF16, tag="yIp")
        yII = pY.tile([P, FW], BF16, tag="yII")
        vpass(I_bf, yI, "dve")
        vpass(p_bf, yp, "act")
        vpass(Ip_bf, yIp, "dve")
        vpass(II_bf, yII, "act")

        # stage 2: W-pass per h-chunk j + elementwise -> a, b (bf16)
        a_bf = pAB.tile([P, FW], BF16, tag="abf")
        b_bf = pAB.tile([P, FW], BF16, tag="bbf")
        for j in range(NB):
            qI = pQ.tile([P, HW_], F32, tag="qI")
            qp = pQ.tile([P, HW_], F32, tag="qp")
            qIp = pQ.tile([P, HW_], F32, tag="qIp")
            qII = pQ.tile([P, HW_], F32, tag="qII")
            wpass_mm(yI, qI, j)
            wpass_mm(yp, qp, j)
            wpass_mm(yIp, qIp, j)
            wpass_mm(yII, qII, j, add_eps=True)

            sl = slice(j * HW_, (j + 1) * HW_)
            cp = pT.tile([P, HW_], F32, tag="cp")
            v = pT.tile([P, HW_], F32, tag="v")
            u = pT.tile([P, HW_], F32, tag="u")
            cov = pT.tile([P, HW_], F32, tag="cov")
            den = pT.tile([P, HW_], F32, tag="den")
            rcp = pT.tile([P, HW_], F32, tag="rcp")
            tt = pT.tile([P, HW_], BF16, tag="tt")
            nc.scalar.copy(cp[:], qp[:])           # mean_p -> SBUF
            nc.scalar.square(v[:], qI[:])          # mean_I^2
            nc.vector.tensor_mul(u[:], cp[:], qI[:])
            nc.vector.tensor_sub(cov[:], qIp[:], u[:])
            nc.vector.tensor_sub(den[:], qII[:], v[:])
            nc.vector.reciprocal_approx_fast(rcp[:], den[:])
            nc.vector.tensor_mul(a_bf[:, sl], cov[:], rcp[:])
            nc.vector.tensor_mul(tt[:], a_bf[:, sl], qI[:])
            nc.vector.tensor_sub(b_bf[:, sl], cp[:], tt[:])

        # stage 3: box2d of a and b, final combine
        ya = pY.tile([P, FW], BF16, tag="ya")
        yb = pY.tile([P, FW], BF16, tag="yb")
        vpass(a_bf, ya, "dve")
        vpass(b_bf, yb, "act")

        out_t = pOut.tile([P, FW], F32, tag="out")
        for j in range(NB):
            ra = pRR.tile([P, HW_], F32, tag="ra")
            rb = pRR.tile([P, HW_], F32, tag="rb")
            wpass_mm(ya, ra, j)
            wpass_mm(yb, rb, j)
            sl = slice(j * HW_, (j + 1) * HW_)
            f1 = pT.tile([P, HW_], F32, tag="f1")
            nc.vector.tensor_mul(f1[:], I_f[:, sl], ra[:])
            nc.vector.tensor_add(out_t[:, sl], rb[:], f1[:])

        nc.sync.dma_start(_img_view(out_d, c), _sb3(out_t))


_NC_CACHE = None
LAST_RESULT = None


def _get_model():
    global _NC_CACHE
    if _NC_CACHE is None:
        _maybe_patch_ldw_opt()
        _NC_CACHE = build_model()
    return _NC_CACHE


def kernel(I, p):
    global LAST_RESULT
    I = np.asarray(I, dtype=np.float32)
    p = np.asarray(p, dtype=np.float32)
    B = I.shape[0]
    assert I.shape == (B, CH, HW_, HW_), I.shape
    nc = _get_model()
    consts = make_consts()
    in_maps = []
    for k in range(NCORES):
        m = {"I": np.ascontiguousarray(I[k]), "p": np.ascontiguousarray(p[k])}
        m.update(consts)
        in_maps.append(m)
    kwargs = {}
    if os.environ.get("BASS_TRACE_DIR"):
        kwargs["tmpdir"] = os.environ["BASS_TRACE_DIR"]
    res = run_bass_kernel_spmd(nc, in_maps, core_ids=list(range(NCORES)), **kwargs)
    LAST_RESULT = res
    out = np.stack([res.results[k]["out"] for k in range(NCORES)], axis=0)
    return out.astype(np.float32)


if __name__ == "__main__":
    rng = np.random.default_rng(0)
    I = rng.random((8, CH, HW_, HW_), dtype=np.float32)
    p = rng.random((8, CH, HW_, HW_), dtype=np.float32)
    out = kernel(I, p)
    print("out", out.shape, out.dtype, float(out.mean()))


# revision 12
# speedup vs baseline: 1.4757x; 1.0796x over previous
"""Guided filter (r=40, eps=1e-3) on 8 Trainium2 NeuronCores.

Sharding: pure data-parallel over the batch dim (8 batches -> 8 cores).
Each core processes 3 channel-images of 512x512.

Algorithm per image:
  box2d(x) done as two banded matmuls on the TensorEngine (version "A":
  the image chunk is the stationary operand, the 0/1 band matrix the
  moving operand; contraction runs over the partition dim so each pass
  both box-filters one axis and transposes the layout).
  - V-pass band columns carry 2^round(log2(1/n_h)) (exact in bf16); the
    per-row residual rho_h is applied later as a per-partition scalar.
  - W-pass band columns carry bf16(1/n_w).
  - eps is added to the II box output via a rank-1 (K=1) matmul.
  Elementwise stage on VectorE/ScalarE consuming PSUM directly.
"""

import os
import sys
import numpy as np
import ml_dtypes
from contextlib import ExitStack

sys.path.insert(0, "/opt/trn_rl_repo")

import concourse.bass as bass
import concourse.tile as tile
from concourse import bacc, mybir
from concourse.bass_utils import run_bass_kernel_spmd

F32 = mybir.dt.float32
BF16 = mybir.dt.bfloat16
ALU = mybir.AluOpType

R = 40
EPS = 1e-3
HW_ = 512
NB = 4  # 128-row blocks per axis
CH = 3  # channels per batch
P = 128
NCORES = 8


def _band_range(c):
    n0 = max(0, P * c - R)
    n1 = min(HW_, P * c + P + R)
    return n0, n1


def make_consts():
    idx = np.arange(HW_)
    n1d = (np.minimum(idx + R, HW_ - 1) - np.maximum(idx - R, 0) + 1).astype(np.float64)
    inv_n = 1.0 / n1d
    E = np.round(np.log2(inv_n))
    po2 = 2.0 ** E                      # exact in bf16
    rho = (inv_n * 2.0 ** (-E)).astype(np.float32)   # residual, ~[0.7, 1.42]

    mask = (np.abs(idx[:, None] - idx[None, :]) <= R)
    bandV = (mask * po2[None, :]).astype(ml_dtypes.bfloat16)
    bandW = (mask * inv_n[None, :]).astype(ml_dtypes.bfloat16)
    # [512k, 512n] -> [128 kp, 4*512 (kb, n)]
    bandV = np.ascontiguousarray(
        bandV.reshape(NB, P, HW_).transpose(1, 0, 2).reshape(P, NB * HW_))
    bandW = np.ascontiguousarray(
        bandW.reshape(NB, P, HW_).transpose(1, 0, 2).reshape(P, NB * HW_))

    rho_t = np.ascontiguousarray(rho.reshape(NB, P).T)          # [128, 4]
    eps2e = (EPS / rho).astype(ml_dtypes.bfloat16).reshape(1, HW_)  # [1, 512]
    ones = np.ones((1, HW_), dtype=ml_dtypes.bfloat16)
    return {"bandV": bandV, "bandW": bandW, "rho": rho_t,
            "eps2e": eps2e, "ones": ones}


def _img_view(dram_ap, c):
    # [3, 512, 512] DRAM tensor -> channel c as [128 hp, 4 hb, 512 w]
    return dram_ap[c].rearrange("(hb hp) w -> hp hb w", hp=P)


def _sb3(t):
    # [128, 2048] SBUF tile AP -> [128, 4, 512]
    return t[:].rearrange("p (hb w) -> p hb w", w=HW_)


def build_model():
    nc = bacc.Bacc("TRN2", target_bir_lowering=False, debug=False,
                   num_devices=NCORES)
    I_d = nc.dram_tensor("I", [CH, HW_, HW_], F32, kind="ExternalInput").ap()
    p_d = nc.dram_tensor("p", [CH, HW_, HW_], F32, kind="ExternalInput").ap()
    bandV_d = nc.dram_tensor("bandV", [P, NB * HW_], BF16, kind="ExternalInput").ap()
    bandW_d = nc.dram_tensor("bandW", [P, NB * HW_], BF16, kind="ExternalInput").ap()
    rho_d = nc.dram_tensor("rho", [P, NB], F32, kind="ExternalInput").ap()
    eps2e_d = nc.dram_tensor("eps2e", [1, HW_], BF16, kind="ExternalInput").ap()
    ones_d = nc.dram_tensor("ones", [1, HW_], BF16, kind="ExternalInput").ap()
    out_d = nc.dram_tensor("out", [CH, HW_, HW_], F32, kind="ExternalOutput").ap()

    with tile.TileContext(nc) as tc:
        with ExitStack() as ctx:
            build_kernel(ctx, tc, I_d, p_d, out_d,
                         bandV_d, bandW_d, rho_d, eps2e_d, ones_d)
    nc.compile()
    return nc


def build_kernel(ctx, tc, I_d, p_d, out_d, bandV_d, bandW_d, rho_d,
                 eps2e_d, ones_d):
    nc = tc.nc
    FW = NB * HW_  # 2048

    consts = ctx.enter_context(tc.tile_pool(name="consts", bufs=1))
    bandV = consts.tile_from(bandV_d)
    bandW = consts.tile_from(bandW_d)
    rho = consts.tile_from(rho_d)
    eps2e = consts.tile_from(eps2e_d)
    ones = consts.tile_from(ones_d)

    # image-grain pools (double-buffered across the 3 channels)
    pIf = ctx.enter_context(tc.tile_pool(name="If", bufs=2))
    pPf = ctx.enter_context(tc.tile_pool(name="Pf", bufs=2))
    pBf = ctx.enter_context(tc.tile_pool(name="ibf", bufs=1))
    pY = ctx.enter_context(tc.tile_pool(name="ymid", bufs=2))
    pM = ctx.enter_context(tc.tile_pool(name="means", bufs=1))
    pAB = ctx.enter_context(tc.tile_pool(name="ab", bufs=2))
    pOut = ctx.enter_context(tc.tile_pool(name="outp", bufs=2))
    pT = ctx.enter_context(tc.tile_pool(name="tmps", bufs=2))
    pV = ctx.enter_context(tc.tile_pool(name="psv", bufs=2, space="PSUM"))
    pQ = ctx.enter_context(tc.tile_pool(name="psq", bufs=1, space="PSUM"))
    pRR = ctx.enter_context(tc.tile_pool(name="psr", bufs=1, space="PSUM"))

    def vpass(src_bf, band, psum_pool, dst_bf, copy_eng="act"):
        """One banded pass: src [h|w] bf16 -> dst [w|h] bf16 (box over
        partition axis + transpose). 16 MMs + 4 PSUM->SBUF copies."""
        for i in range(NB):
            ps = psum_pool.tile([P, HW_], F32, tag="ps")
            for j in range(NB):
                n0, n1 = _band_range(j)
                nc.tensor.matmul(
                    ps[:, n0:n1],
                    lhsT=src_bf[:, j * HW_ + i * P: j * HW_ + i * P + P],
                    rhs=band[:, j * HW_ + n0: j * HW_ + n1],
                    start=(j == 0), stop=(j == NB - 1))
            if copy_eng == "dve":
                nc.vector.tensor_copy(dst_bf[:, i * HW_:(i + 1) * HW_], ps[:])
            else:
                nc.scalar.copy(dst_bf[:, i * HW_:(i + 1) * HW_], ps[:])

    def wpass_mm(src_bf, band, q_tile, j, add_eps=False):
        """W-direction banded MMs for output h-chunk j into q_tile."""
        for i in range(NB):
            m0, m1 = _band_range(i)
            last = (i == NB - 1) and not add_eps
            nc.tensor.matmul(
                q_tile[:, m0:m1],
                lhsT=src_bf[:, i * HW_ + j * P: i * HW_ + j * P + P],
                rhs=band[:, i * HW_ + m0: i * HW_ + m1],
                start=(i == 0), stop=last)
        if add_eps:
            nc.tensor.matmul(
                q_tile[:, :],
                lhsT=eps2e[:1, j * P:(j + 1) * P],
                rhs=ones[:1, :],
                start=False, stop=True)

    for c in range(CH):
        I_f = pIf.tile([P, FW], F32, tag="If")
        p_f = pPf.tile([P, FW], F32, tag="pf")
        nc.sync.dma_start(_sb3(I_f), _img_view(I_d, c))
        nc.sync.dma_start(_sb3(p_f), _img_view(p_d, c))

        I_bf = pBf.tile([P, FW], BF16, tag="Ibf")
        p_bf = pBf.tile([P, FW], BF16, tag="pbf")
        Ip_bf = pBf.tile([P, FW], BF16, tag="Ipbf")
        II_bf = pBf.tile([P, FW], BF16, tag="IIbf")
        nc.scalar.copy(I_bf[:], I_f[:])
        nc.scalar.copy(p_bf[:], p_f[:])
        nc.vector.tensor_mul(Ip_bf[:], I_bf[:], p_bf[:])
        nc.scalar.square(II_bf[:], I_f[:])

        # stage 1: V-pass for the four tensors -> [w|h] bf16 mids
        yI = pY.tile([P, FW], BF16, tag="yI")
        yp = pY.tile([P, FW], BF16, tag="yp")
        yIp = pY.tile([P, FW], BF16, tag="yIp")
        yII = pY.tile([P, FW], BF16, tag="yII")
        vpass(I_bf, bandV, pV, yI, "act")
        vpass(p_bf, bandV, pV, yp, "act")
        vpass(Ip_bf, bandV, pV, yIp, "act")
        vpass(II_bf, bandV, pV, yII, "dve")

        # stage 2: W-pass per h-chunk j + elementwise -> a, b (bf16)
        mI = pM.tile([P, FW], F32, tag="mI")
        mp = pM.tile([P, FW], F32, tag="mp")
        a_bf = pAB.tile([P, FW], BF16, tag="abf")
        b_bf = pAB.tile([P, FW], BF16, tag="bbf")
        for j in range(NB):
            qI = pQ.tile([P, HW_], F32, tag="qI")
            qp = pQ.tile([P, HW_], F32, tag="qp")
            qIp = pQ.tile([P, HW_], F32, tag="qIp")
            qII = pQ.tile([P, HW_], F32, tag="qII")
            wpass_mm(yI, bandW, qI, j)
            wpass_mm(yp, bandW, qp, j)
            wpass_mm(yIp, bandW, qIp, j)
            wpass_mm(yII, bandW, qII, j, add_eps=True)

            s = rho[:, j:j + 1]
            sl = slice(j * HW_, (j + 1) * HW_)
            mIj = mI[:, sl]
            mpj = mp[:, sl]
            nc.scalar.mul(mIj, qI[:], s)          # mean_I (frees qI)
            nc.scalar.mul(mpj, qp[:], s)          # mean_p (frees qp)
            u = pT.tile([P, HW_], F32, tag="u")
            cov = pT.tile([P, HW_], F32, tag="cov")
            v = pT.tile([P, HW_], F32, tag="v")
            den = pT.tile([P, HW_], F32, tag="den")
            rcp = pT.tile([P, HW_], F32, tag="rcp")
            tt = pT.tile([P, HW_], BF16, tag="tt")
            nc.scalar.square(v[:], mIj)
            nc.vector.scalar_tensor_tensor(
                den[:], qII[:], s, v[:], op0=ALU.mult, op1=ALU.subtract)
            nc.vector.tensor_mul(u[:], mIj, mpj)
            nc.vector.scalar_tensor_tensor(
                cov[:], qIp[:], s, u[:], op0=ALU.mult, op1=ALU.subtract)
            nc.vector.reciprocal_approx_fast(rcp[:], den[:])
            nc.vector.tensor_mul(a_bf[:, sl], cov[:], rcp[:])
            nc.vector.tensor_mul(tt[:], a_bf[:, sl], mIj)
            nc.vector.tensor_sub(b_bf[:, sl], mpj, tt[:])

        # stage 3: box2d of a and b, final combine
        ya = pY.tile([P, FW], BF16, tag="ya")
        yb = pY.tile([P, FW], BF16, tag="yb")
        vpass(a_bf, bandV, pV, ya, "dve")
        vpass(b_bf, bandV, pV, yb, "act")

        out_t = pOut.tile([P, FW], F32, tag="out")
        for j in range(NB):
            ra = pRR.tile([P, HW_], F32, tag="ra")
            rb = pRR.tile([P, HW_], F32, tag="rb")
            wpass_mm(ya, bandW, ra, j)
            wpass_mm(yb, bandW, rb, j)
            s = rho[:, j:j + 1]
            sl = slice(j * HW_, (j + 1) * HW_)
            f1 = pT.tile([P, HW_], F32, tag="f1")
            nc.vector.scalar_tensor_tensor(
                f1[:], ra[:], s, I_f[:, sl], op0=ALU.mult, op1=ALU.mult)
            nc.vector.scalar_tensor_tensor(
                out_t[:, sl], rb[:], s, f1[:], op0=ALU.mult, op1=ALU.add)

        nc.sync.dma_start(_img_view(out_d, c), _sb3(out_t))


_NC_CACHE = None
LAST_RESULT = None


def _get_model():
    global _NC_CACHE
    if _NC_CACHE is None:
        _NC_CACHE = build_model()
    return _NC_CACHE


def kernel(I, p):
    global LAST_RESULT
    I = np.asarray(I, dtype=np.float32)
    p = np.asarray(p, dtype=np.float32)
    B = I.shape[0]
    assert I.shape == (B, CH, HW_, HW_), I.shape
    nc = _get_model()
    consts = make_consts()
    in_maps = []
    for k in range(NCORES):
        m = {"I": np.ascontiguousarray(I[k]), "p": np.ascontiguousarray(p[k])}
        m.update(consts)
        in_maps.append(m)
    kwargs = {}
    if os.environ.get("BASS_TRACE_DIR"):
        kwargs["tmpdir"] = os.environ["BASS_TRACE_DIR"]
    res = run_bass_kernel_spmd(nc, in_maps, core_ids=list(range(NCORES)), **kwargs)
    LAST_RESULT = res
    out = np.stack([res.results[k]["out"] for k in range(NCORES)], axis=0)
    return out.astype(np.float32)


if __name__ == "__main__":
    rng = np.random.default_rng(0)
    I = rng.random((8, CH, HW_, HW_), dtype=np.float32)
    p = rng.random((8, CH, HW_, HW_), dtype=np.float32)
    out = kernel(I, p)
    print("out", out.shape, out.dtype, float(out.mean()))

